# revision 6
# baseline (speedup 1.0000x reference)
"""Trainium2 Bass kernel for nn_AttentionBlock (GroupNorm + 1x1-conv attention).

Contract: kernel(**inputs) takes FULL unsharded inputs (numpy, shapes as in
setup_inputs) and returns the FULL output. Internally shards batch (32) over
8 NeuronCores (4 batch elements per core), params replicated.

FP8 DoubleRow version: all five matmul phases (q/k/v projections, scores,
P@V, output projection) run as fp8e4 DoubleRow matmuls (2 fp8 weights per PE
cell, K=256 contraction per instruction, ~1.5x bf16 throughput).  The
residual path (+x) stays bf16/f32, which keeps the overall rel-err ~1e-2
despite ~2% fp8 quantization on the attention path (attention output is
~0.1x the residual magnitude).

Scaling scheme (fp8e4 subnormal boundary is 2^-6; weights are ~N(0,1/512)):
  wq,wk,wv are pre-scaled x8 on the host; bq,bk,bv x8 on device.  So
  q2/k2 = 8(q+b), vT = 8(v+b).  The score psum is 64x -> exp scale /64.
  exp uses offset -3 (cancels in Z).  PV psum = 8*Z*O; zinv = 1/(8Z)
  recovers O exactly.  wo is unscaled.

Z (softmax denominators) is computed with ones-stationary DoubleRow matmuls
into a [1, 512] psum row, then transposed to per-partition columns via a
tiny DRAM round-trip (tile-tracked).  The S phase is at-major so each
half's Z round-trip overlaps the next phase's matmuls.
"""

import sys

sys.path.insert(0, "/opt/trn_rl_repo")

from contextlib import ExitStack

import numpy as np

import concourse.bass as bass
import concourse.tile as tile
from concourse import bacc, mybir
from concourse.bass_utils import run_bass_kernel_spmd

B, H, W, C = 32, 32, 32, 512
HW = H * W  # 1024
NCORES = 8
NB = B // NCORES  # 4 batch elements per core
P = 128
GROUPS = 32
EPS = 1e-6
F32 = mybir.dt.float32
BF16 = mybir.dt.bfloat16
F8 = mybir.dt.float8e4
F8E5 = mybir.dt.float8e5  # for P=exp(s-3): e4m3 tops out at 240 = exp(8.48),
                          # and the data's max score is ~9.0; e5m2 caps at 57344.

CT = C // P  # 4 channel tiles
MT = HW // P  # 8 pixel tiles
KP = CT // 2  # 2 DoubleRow channel-pair steps
WS = 8.0  # host pre-scale on wq/wk/wv; device scale on bq/bk/bv
EXP_OFF = -3.0
DR = mybir.MatmulPerfMode.DoubleRow


def build_bass(nb: int = NB):
    nc = bacc.Bacc()

    x_in = nc.declare_dram_parameter("xbf16", [nb, HW, C], BF16, isOutput=False)
    gamma_in = nc.declare_dram_parameter("gn_gamma", [C], F32, isOutput=False)
    beta_in = nc.declare_dram_parameter("gn_beta", [C], F32, isOutput=False)
    wq_in = nc.declare_dram_parameter("wq", [C, C], F8, isOutput=False)
    bq_in = nc.declare_dram_parameter("bq", [C], F32, isOutput=False)
    wk_in = nc.declare_dram_parameter("wk", [C, C], F8, isOutput=False)
    bk_in = nc.declare_dram_parameter("bk", [C], F32, isOutput=False)
    wv_in = nc.declare_dram_parameter("wv", [C, C], F8, isOutput=False)
    bv_in = nc.declare_dram_parameter("bv", [C], F32, isOutput=False)
    wo_in = nc.declare_dram_parameter("wo", [C, C], F8, isOutput=False)
    bo_in = nc.declare_dram_parameter("bo", [C], F32, isOutput=False)
    out_ext = nc.declare_dram_parameter("out", [nb, HW, C], BF16, isOutput=True)

    # Block-diagonal group-averaging matrix: gmat[i, j] = 1/16 iff same group.
    gs = C // GROUPS  # 16 channels per group
    gnp = np.zeros((P, P), dtype=np.float32)
    for g in range(P // gs):
        gnp[g * gs : (g + 1) * gs, g * gs : (g + 1) * gs] = 1.0 / gs
    gmat_dram = nc.inline_tensor(gnp, name="gmat")

    inv_sqrt_c = float(C) ** -0.5
    exp_scale = inv_sqrt_c / (WS * WS)

    with tile.TileContext(nc) as tc, ExitStack() as ctx:
        ep = ctx.enter_context

        consts = ep(tc.tile_pool(name="consts", bufs=1))
        p_xb = ep(tc.tile_pool(name="p_xb", bufs=2))      # bf16 x residual
        p_xT = ep(tc.tile_pool(name="p_xT", bufs=CT + 2))
        p_xn = ep(tc.tile_pool(name="p_xn", bufs=2))
        p_st = ep(tc.tile_pool(name="p_st", bufs=4))
        p_qk = ep(tc.tile_pool(name="p_qk", bufs=4))
        p_pt = ep(tc.tile_pool(name="p_pt", bufs=2))
        p_v = ep(tc.tile_pool(name="p_v", bufs=2))
        p_op = ep(tc.tile_pool(name="p_op", bufs=2))
        p_z = ep(tc.tile_pool(name="p_z", bufs=8))
        p_out = ep(tc.tile_pool(name="p_out", bufs=6))
        p_zd = ep(tc.tile_pool(name="p_zd", bufs=4, space="DRAM"))

        # PSUM: 8 banks.  pp(2) + ps(2) + po(2) + psm(1) + pz(1) = 8
        pp = ep(tc.tile_pool(name="pp", bufs=2, space="PSUM"))
        ps = ep(tc.tile_pool(name="ps", bufs=2, space="PSUM"))
        po = ep(tc.tile_pool(name="po", bufs=2, space="PSUM"))
        psm = ep(tc.tile_pool(name="psm", bufs=1, space="PSUM"))
        pz = ep(tc.tile_pool(name="pz", bufs=1, space="PSUM"))

        # ---- small constants first so GroupNorm of elem 0 can start early ----
        gcol = consts.tile([P, CT], F32, name="gamma")
        nc.sync.dma_start(gcol, gamma_in.rearrange("(t p) -> p t", p=P))
        bcol = consts.tile([P, CT], F32, name="beta")
        nc.sync.dma_start(bcol, beta_in.rearrange("(t p) -> p t", p=P))
        bv_col8 = consts.tile([P, CT], F32, name="bv8")
        nc.sync.dma_start(bv_col8, bv_in.rearrange("(t p) -> p t", p=P))
        nc.vector.tensor_scalar_mul(bv_col8, bv_col8, WS)
        gmat_sb = consts.tile([P, P], F32, name="gmat")
        nc.sync.dma_start(gmat_sb, gmat_dram[:, :])
        eps_sb = consts.tile([P, 1], F32, name="eps")
        nc.vector.memset(eps_sb, EPS)
        neg3_sb = consts.tile([P, 1], F32, name="neg3")
        nc.vector.memset(neg3_sb, EXP_OFF)
        ones8 = consts.tile([P, 2, 16], F8, name="ones8")
        nc.vector.memset(ones8, 1.0)

        w_sb = {}

        for ib in range(nb):
            # pixel-major view of this element's x slab, (128, 8, 512)-tiled
            xb_v = x_in[ib].rearrange("(t p) c -> p t c", p=P)

            # ---- transpose-load x^T straight from the DRAM input ----
            xT = []
            for ct in range(CT):
                tt = p_xT.tile([P, HW], BF16, name="xT")
                nc.sync.dma_start_transpose(tt, x_in[ib][:, ct * P : (ct + 1) * P])
                xT.append(tt)

            if ib == 0:
                # fp8 weights + f32 broadcast row biases (x8 for q/k)
                for name, wext in (
                    ("q", wq_in), ("k", wk_in), ("v", wv_in), ("o", wo_in)
                ):
                    wb = consts.tile([P, CT, C], F8, name=f"w_{name}")
                    nc.sync.dma_start(wb, wext.rearrange("(kt p) c -> p kt c", p=P))
                    w_sb[name] = wb
                bq8 = consts.tile([P, C], F32, name="bq8")
                nc.sync.dma_start(bq8, bq_in[None, :].to_broadcast((P, C)))
                nc.vector.tensor_scalar_mul(bq8, bq8, WS)
                bk8 = consts.tile([P, C], F32, name="bk8")
                nc.sync.dma_start(bk8, bk_in[None, :].to_broadcast((P, C)))
                nc.vector.tensor_scalar_mul(bk8, bk8, WS)
                bo_sb = consts.tile([P, C], F32, name="bo")
                nc.sync.dma_start(bo_sb, bo_in[None, :].to_broadcast((P, C)))

            # bf16 x kept in SBUF for the final residual add
            xallb = p_xb.tile([P, MT, C], BF16, name="xallb")
            nc.sync.dma_start(xallb, xb_v)

            # ---- GroupNorm -> xn_pix fp8 [P, kt, pix] (channel-tile major) ----
            xn_pix = p_xn.tile([P, CT, HW], F8, name="xn_pix")
            for ct in range(CT):
                stats = p_st.tile([P, 2, 6], F32, name="bnstats")
                nc.vector.bn_stats(stats[:, 0, :], xT[ct][:, 0:512])
                nc.vector.bn_stats(stats[:, 1, :], xT[ct][:, 512:1024])
                mv = p_st.tile([P, 2], F32, name="mv")
                nc.vector.bn_aggr(mv, stats)
                # msq = [mean_ch, var_ch + mean_ch^2] = [mean_ch, E[x^2]_ch]
                msq = p_st.tile([P, 2], F32, name="msq")
                nc.vector.tensor_copy(msq[:, 0:1], mv[:, 0:1])
                nc.vector.tensor_mul(msq[:, 1:2], mv[:, 0:1], mv[:, 0:1])
                nc.vector.tensor_add(msq[:, 1:2], msq[:, 1:2], mv[:, 1:2])
                # group-average across the 16 channels of each group
                gps = psm.tile([P, 2], F32, name="gps")
                nc.tensor.matmul(gps, lhsT=gmat_sb, rhs=msq, start=True, stop=True)
                mu = p_st.tile([P, 1], F32, name="mu")
                nc.vector.tensor_copy(mu, gps[:, 0:1])
                varg = p_st.tile([P, 1], F32, name="varg")
                nc.vector.tensor_mul(varg, mu, mu)
                nc.vector.tensor_tensor(
                    varg, gps[:, 1:2], varg, mybir.AluOpType.subtract
                )
                sd = p_st.tile([P, 1], F32, name="sd")
                nc.scalar.activation(
                    sd, varg, mybir.ActivationFunctionType.Sqrt, bias=eps_sb[:, 0:1]
                )
                nc.vector.reciprocal(sd, sd)
                scale_col = p_st.tile([P, 1], F32, name="scale_col")
                nc.vector.tensor_mul(scale_col, sd, gcol[:, ct : ct + 1])
                shift_col = p_st.tile([P, 1], F32, name="shift_col")
                nc.vector.tensor_mul(shift_col, mu, scale_col)
                nc.vector.tensor_tensor(
                    shift_col, bcol[:, ct : ct + 1], shift_col, mybir.AluOpType.subtract
                )
                nc.gpsimd.tensor_scalar(
                    out=xn_pix[:, ct, :],
                    in0=xT[ct],
                    scalar1=scale_col,
                    scalar2=shift_col,
                    op0=mybir.AluOpType.mult,
                    op1=mybir.AluOpType.add,
                )

            # stride-2 pixel view for the q/k stationary operand:
            # pixel = rt*256 + 2m + u
            xnv = xn_pix.rearrange("p kt (rt m u) -> p kt rt u m", rt=CT, u=2)

            # ---- q, k projections in the raw-reshape (Q2/K2) layout, x8 ----
            q2sb = p_qk.tile([P, CT, HW], F8, name="q2")
            k2sb = p_qk.tile([P, CT, HW], F8, name="k2")
            for rt in range(CT):
                for u in range(2):
                    for big, wname, brow in ((q2sb, "q", bq8), (k2sb, "k", bk8)):
                        acc = pp.tile([P, C], F32, name="proj_ps")
                        for j in range(KP):
                            nc.tensor.matmul(
                                acc,
                                lhsT=xnv[:, 2 * j : 2 * j + 2, rt, u, :],
                                rhs=w_sb[wname][:, 2 * j : 2 * j + 2, :],
                                start=(j == 0),
                                stop=(j == KP - 1),
                                perf_mode=DR,
                            )
                        nc.vector.tensor_add(
                            big[:, rt, u * 512 : (u + 1) * 512], acc, brow
                        )

            # ---- v projection -> vT = 8*(V2^T + bias): [P, bt, i] fp8 ----
            # bt 0..3: even pixels (V2 cols 0..511), bt 4..7: odd pixels.
            vT = p_v.tile([P, MT, 512], F8, name="vT")
            for ct in range(CT):
                for n in range(2):
                    acc = pp.tile([P, 512], F32, name="proj_ps")
                    for j in range(KP):
                        nc.tensor.matmul(
                            acc,
                            lhsT=w_sb["v"][:, 2 * j : 2 * j + 2, ct * P : (ct + 1) * P],
                            rhs=xn_pix[:, 2 * j : 2 * j + 2, n * 512 : (n + 1) * 512],
                            start=(j == 0),
                            stop=(j == KP - 1),
                            perf_mode=DR,
                        )
                    pv = acc.rearrange("p (m u) -> p u m", u=2)
                    nc.scalar.activation(
                        vT[:, ct, n * 256 : (n + 1) * 256],
                        pv[:, 0, :],
                        mybir.ActivationFunctionType.Identity,
                        bias=bv_col8[:, ct : ct + 1],
                    )
                    nc.scalar.activation(
                        vT[:, CT + ct, n * 256 : (n + 1) * 256],
                        pv[:, 1, :],
                        mybir.ActivationFunctionType.Identity,
                        bias=bv_col8[:, ct : ct + 1],
                    )

            # ---- S^T = K2^T Q2 (x64), P^T = exp(S^T/(64 sqrt c) - 3) fp8;
            # at-major so each half's Z row->col round-trip overlaps compute ----
            PT = p_pt.tile([P, MT, HW], F8E5, name="pt")
            zcol = p_z.tile([P, MT], F32, name="zcol")
            zdram = p_zd.tile([2, 512], F32, name="zdram")
            for at in range(2):
                for bt in range(MT):
                    sps = ps.tile([P, 512], F32, name="s_ps")
                    for j in range(KP):
                        nc.tensor.matmul(
                            sps,
                            lhsT=k2sb[:, 2 * j : 2 * j + 2, bt * P : (bt + 1) * P],
                            rhs=q2sb[:, 2 * j : 2 * j + 2, at * 512 : (at + 1) * 512],
                            start=(j == 0),
                            stop=(j == KP - 1),
                            perf_mode=DR,
                        )
                    nc.scalar.activation(
                        PT[:, bt, at * 512 : (at + 1) * 512],
                        sps,
                        mybir.ActivationFunctionType.Exp,
                        bias=neg3_sb[:, 0:1],
                        scale=exp_scale,
                    )
                # Z for this half: ones-stationary matmul -> [1, 512] psum row,
                # x8, then DRAM round-trip to per-partition columns.
                zps = pz.tile([1, 512], F32, name="z_ps")
                for j in range(MT // 2):
                    nc.tensor.matmul(
                        zps,
                        lhsT=ones8[:, :, 0:1],
                        rhs=PT[:, 2 * j : 2 * j + 2, at * 512 : (at + 1) * 512],
                        start=(j == 0),
                        stop=(j == MT // 2 - 1),
                        perf_mode=DR,
                    )
                zrow = p_z.tile([1, 512], F32, name="zrow")
                nc.vector.tensor_scalar_mul(zrow, zps, WS)  # 8*Z
                nc.sync.dma_start(zdram[at], zrow)
                nc.sync.dma_start(
                    zcol[:, at * (MT // 2) : (at + 1) * (MT // 2)],
                    zdram[at].rearrange("(t p) -> p t", p=P),
                )
            zinv = p_z.tile([P, MT], F32, name="zinv")
            nc.vector.reciprocal(zinv, zcol)  # 1/(8Z)

            # ---- O^T[a, i] = sum_b P^T[b, a] * vT[b, i]; drain * zinv undoes
            # the raw reshape into opT (X^T layout, fp8) ----
            opT = p_op.tile([P, CT, HW], F8, name="opT")
            for am in range(MT):
                ops = po.tile([P, 512], F32, name="o_ps")
                for j in range(MT // 2):
                    nc.tensor.matmul(
                        ops,
                        lhsT=PT[:, 2 * j : 2 * j + 2, am * P : (am + 1) * P],
                        rhs=vT[:, 2 * j : 2 * j + 2, :],
                        start=(j == 0),
                        stop=(j == MT // 2 - 1),
                        perf_mode=DR,
                    )
                cht, u = am % CT, am // CT
                dst = opT[:, cht].rearrange("p (m u) -> p u m", u=2)[:, u, :]
                nc.vector.tensor_scalar_mul(dst, ops, zinv[:, am : am + 1])

            # ---- final projection + bias + residual ----
            for mt in range(MT):
                acc = pp.tile([P, C], F32, name="proj_ps")
                for j in range(KP):
                    nc.tensor.matmul(
                        acc,
                        lhsT=opT[:, 2 * j : 2 * j + 2, mt * P : (mt + 1) * P],
                        rhs=w_sb["o"][:, 2 * j : 2 * j + 2, :],
                        start=(j == 0),
                        stop=(j == KP - 1),
                        perf_mode=DR,
                    )
                osb = p_out.tile([P, C], BF16, name="osb")
                nc.vector.tensor_add(osb, acc, bo_sb)
                osb2 = p_out.tile([P, C], BF16, name="osb2")
                nc.gpsimd.tensor_add(osb2, osb, xallb[:, mt, :])
                nc.sync.dma_start(out_ext[ib, mt * P : (mt + 1) * P, :], osb2)

    nc.finalize()
    return nc


_nc_cache = {}


def get_nc(nb: int = NB):
    if nb not in _nc_cache:
        _nc_cache[nb] = build_bass(nb)
    return _nc_cache[nb]


def kernel(x, gn_gamma, gn_beta, wq, bq, wk, bk, wv, bv, wo, bo, **run_kwargs):
    import ml_dtypes

    bf16 = ml_dtypes.bfloat16
    f8 = ml_dtypes.float8_e4m3
    xb = np.ascontiguousarray(
        np.asarray(x, dtype=np.float32).astype(bf16)
    ).reshape(B, HW, C)

    def w8(w):
        return np.ascontiguousarray(
            (np.asarray(w, dtype=np.float32) * WS).astype(f8)
        )

    params = {
        "gn_gamma": np.ascontiguousarray(np.asarray(gn_gamma, dtype=np.float32)),
        "gn_beta": np.ascontiguousarray(np.asarray(gn_beta, dtype=np.float32)),
        "wq": w8(wq),
        "bq": np.ascontiguousarray(np.asarray(bq, dtype=np.float32)),
        "wk": w8(wk),
        "bk": np.ascontiguousarray(np.asarray(bk, dtype=np.float32)),
        "wv": w8(wv),
        "bv": np.ascontiguousarray(np.asarray(bv, dtype=np.float32)),
        "wo": np.ascontiguousarray(np.asarray(wo, dtype=np.float32).astype(f8)),
        "bo": np.ascontiguousarray(np.asarray(bo, dtype=np.float32)),
    }
    nc = get_nc(NB)
    in_maps = [
        {"xbf16": xb[i * NB : (i + 1) * NB], **params} for i in range(NCORES)
    ]
    res = run_bass_kernel_spmd(nc, in_maps, core_ids=list(range(NCORES)), **run_kwargs)
    global last_results
    last_results = res
    out = np.concatenate([res.results[i]["out"] for i in range(NCORES)], axis=0)
    return out.reshape(B, H, W, C).astype(np.float32)


last_results = None


if __name__ == "__main__":
    nc = build_bass(NB)
    print("build + compile OK")
